# revision 1
# baseline (speedup 1.0000x reference)
"""Sharded GQA attention (causal + packed-segment mask) for 8 Trainium2 NeuronCores.

Strategy
--------
* Core c handles batch b = c//4 and KV heads {2*(c%4), 2*(c%4)+1} (8 query
  heads per core); the sequence dim stays unsharded.
* decoder_segment_ids are sorted, so the segment mask makes attention
  block-diagonal over contiguous segment spans.  The host reads the actual
  ids, splits each batch into runs, and the device kernel does causal-only
  attention per segment.  The two batches' run structures are unioned
  (padded) so all 8 cores execute one SPMD program; padded "ghost" rows are
  masked with per-core additive mask tiles and ghost query columns produce
  garbage that the host discards on re-assembly.
* Per (segment, kv, t-block): S^T[s, (g,t)] tiles are built by PE matmuls
  (K-chunk stationary [d,s], Q^T moving [d, 4*128]); causal/ghost masking is
  an identity-stationary matmul accumulating a host-built additive mask
  (mask matmuls run as float32r -- products are exact 1*M/0*M, and fp32r
  streams 4x faster through the PE than fp32); exp runs on ScalarE straight
  out of PSUM (no max subtraction -- logits are bounded, fp32-safe); PV uses
  P^T tiles as stationary against V chunks padded to 130 columns with an
  appended ones column so the softmax denominator falls out of the same
  matmuls; the final normalize is a reciprocal + broadcast tensor_tensor
  multiply on DVE fused with the PSUM->SBUF copy.  QK/PV matmuls stay plain
  fp32: float32r (TF32-class rounding) measured 2.3x faster end-to-end but
  cost 1.9e-3 relative error vs the fp32 reference; fp32 keeps it at 6.5e-6.

Measured on the 8 axon-tunneled trn2 NeuronCores (For_i-looped timing):
  ~95.1 us per invocation, relative error 6.5e-06.
"""

import math

import numpy as np

B, T, NQ, NKV, D = 2, 1024, 32, 8, 128
G = NQ // NKV
NCORES = 8
KV_PER_CORE = NKV // (NCORES // B)
CHUNK = 128
NEG = -1.0e9
GRP = 1  # s-chunks per PSUM slab (1 bank, 4-deep pipelined)

_PROGRAM_CACHE = {}


# --------------------------------------------------------------------------
# host-side structure
# --------------------------------------------------------------------------

def _runs(seg_row):
    d = np.flatnonzero(np.diff(seg_row) != 0)
    starts = np.concatenate(([0], d + 1))
    ends = np.concatenate((d + 1, [len(seg_row)]))
    return [(int(s), int(e - s)) for s, e in zip(starts, ends)]


def _structure(ids):
    runs = [_runs(np.asarray(ids[b])) for b in range(B)]
    n_seg = max(len(r) for r in runs)
    L = [max((r[i][1] for r in runs if len(r) > i), default=0) for i in range(n_seg)]
    K = [math.ceil(l / CHUNK) for l in L]
    slots = set()
    for i in range(n_seg):
        if K[i] == 0:
            continue
        ghost = set()
        for b in range(B):
            lb = runs[b][i][1] if len(runs[b]) > i else 0
            for c in range(lb // CHUNK, K[i]):
                ghost.add(c)
        for j in range(K[i]):
            for c in range(j + 1):
                if c == j or c in ghost:
                    slots.add((i, c, j))
    slots = sorted(slots)
    segs = [i for i in range(n_seg) if K[i] > 0]
    slabs = [(i, kv_i, j) for i in segs for kv_i in range(KV_PER_CORE)
             for j in range(K[i])]
    chunks = [(i, kv_i, c) for i in segs for kv_i in range(KV_PER_CORE)
              for c in range(K[i])]
    return runs, L, K, slots, segs, slabs, chunks


def _prepare_core(core, q, k, v, runs, L, K, slots, segs, slabs, chunks):
    b = core // (NCORES // B)
    kv_heads = [KV_PER_CORE * (core % (NCORES // B)) + x for x in range(KV_PER_CORE)]
    rb = runs[b]

    def seg_info(i):
        if i < len(rb):
            return rb[i]
        return (0, 0)

    qT = np.zeros((D, len(slabs) * 4 * CHUNK), np.float32)
    for si, (i, kv_i, j) in enumerate(slabs):
        a, lb = seg_info(i)
        t0 = j * CHUNK
        n_real = min(CHUNK, lb - t0)
        if n_real > 0:
            for g in range(G):
                h = G * kv_heads[kv_i] + g
                blk = q[b, a + t0:a + t0 + n_real, h, :]  # [n_real, D]
                qT[:, si * 512 + g * CHUNK: si * 512 + g * CHUNK + n_real] = blk.T

    kT = np.zeros((D, len(chunks) * CHUNK), np.float32)
    vO = np.zeros((CHUNK, len(chunks) * 130), np.float32)
    for ci, (i, kv_i, c) in enumerate(chunks):
        a, lb = seg_info(i)
        s0 = c * CHUNK
        n_real = min(CHUNK, lb - s0)
        if n_real > 0:
            kvh = kv_heads[kv_i]
            kT[:, ci * CHUNK: ci * CHUNK + n_real] = k[b, a + s0:a + s0 + n_real, kvh, :].T
            vO[:n_real, ci * 130: ci * 130 + D] = v[b, a + s0:a + s0 + n_real, kvh, :]
            vO[:n_real, ci * 130 + D] = 1.0

    sr = np.arange(CHUNK)
    m4 = np.zeros((CHUNK, max(len(slots), 1) * 512), np.float32)
    for mi, (i, c, j) in enumerate(slots):
        _, lb = seg_info(i)
        srow = c * CHUNK + sr
        tcol = j * CHUNK + sr
        m = np.where((srow[:, None] > tcol[None, :]) | (srow[:, None] >= lb),
                     np.float32(NEG), np.float32(0.0))
        m4[:, mi * 512:(mi + 1) * 512] = np.tile(m, (1, G))

    return {"qT": qT, "kT": kT, "vO": vO, "m4": m4,
            "ident": np.eye(CHUNK, dtype=np.float32)}


def _assemble(outs, runs, L, K, slabs):
    full = np.zeros((B, T, NQ, D), np.float32)
    for core in range(NCORES):
        b = core // (NCORES // B)
        kv_heads = [KV_PER_CORE * (core % (NCORES // B)) + x
                    for x in range(KV_PER_CORE)]
        res = outs[core]  # [NSLAB, 128, 512]
        rb = runs[b]
        for si, (i, kv_i, j) in enumerate(slabs):
            if i >= len(rb):
                continue
            a, lb = rb[i]
            t0 = j * CHUNK
            n_real = min(CHUNK, lb - t0)
            if n_real <= 0:
                continue
            for g in range(G):
                h = G * kv_heads[kv_i] + g
                full[b, a + t0:a + t0 + n_real, h, :] = \
                    res[si, :n_real, g * CHUNK:g * CHUNK + D]
    return full


# --------------------------------------------------------------------------
# numpy emulation of the device schedule (debug/validation only)
# --------------------------------------------------------------------------

def _numpy_schedule(ins, L, K, slots, segs, slabs, chunks):
    slab_idx = {s: i for i, s in enumerate(slabs)}
    chunk_idx = {c: i for i, c in enumerate(chunks)}
    slot_idx = {s: i for i, s in enumerate(slots)}
    qT, kT, vO, m4 = ins["qT"], ins["kT"], ins["vO"], ins["m4"]
    out = np.zeros((len(slabs), CHUNK, 512), np.float32)
    for i in segs:
        for kv_i in range(KV_PER_CORE):
            for j in range(K[i]):
                si = slab_idx[(i, kv_i, j)]
                ot = np.zeros((CHUNK, G, 129), np.float32)
                for c in range(j + 1):
                    ci = chunk_idx[(i, kv_i, c)]
                    lhsT = kT[:, ci * CHUNK:(ci + 1) * CHUNK]          # [d, s]
                    rhs = qT[:, si * 512:(si + 1) * 512]               # [d, (g,t)]
                    S = lhsT.T @ rhs                                   # [s, (g,t)]
                    if (i, c, j) in slot_idx:
                        mi = slot_idx[(i, c, j)]
                        S = S + m4[:, mi * 512:(mi + 1) * 512]
                    P = np.exp(S)
                    vo = vO[:, ci * 130:ci * 130 + 129]                # [s, 129]
                    for g in range(G):
                        ot[:, g, :] += P[:, g * CHUNK:(g + 1) * CHUNK].T @ vo
                denom = ot[:, :, D:D + 1]
                with np.errstate(divide="ignore", invalid="ignore"):
                    norm = ot[:, :, :D] / denom
                out[si] = norm.reshape(CHUNK, G * D)
    return out


# --------------------------------------------------------------------------
# bass program
# --------------------------------------------------------------------------

def _build_program(L, K, slots, segs, slabs, chunks, loop_n=0, tiny_dma=False):
    import contextlib

    import concourse.bacc as bacc
    import concourse.bass as bass
    import concourse.tile as tile
    from concourse import mybir

    slab_idx = {s: i for i, s in enumerate(slabs)}
    chunk_idx = {c: i for i, c in enumerate(chunks)}
    slot_idx = {s: i for i, s in enumerate(slots)}
    f32 = mybir.dt.float32
    f32r = mybir.dt.float32r

    nc = bacc.Bacc()
    qT_d = nc.dram_tensor("qT", [D, len(slabs) * 512], f32, kind="ExternalInput")
    kT_d = nc.dram_tensor("kT", [D, len(chunks) * CHUNK], f32, kind="ExternalInput")
    vO_d = nc.dram_tensor("vO", [CHUNK, len(chunks) * 130], f32, kind="ExternalInput")
    m4_d = nc.dram_tensor("m4", [CHUNK, max(len(slots), 1) * 512], f32r,
                          kind="ExternalInput")
    id_d = nc.dram_tensor("ident", [CHUNK, CHUNK], f32r, kind="ExternalInput")
    out_d = nc.dram_tensor("out", [len(slabs), CHUNK, 512], f32,
                           kind="ExternalOutput")

    def _dma(eng, out, in_):
        if tiny_dma:
            eng.dma_start(out=out[:1, :4], in_=in_[:1, :4])
        else:
            eng.dma_start(out=out, in_=in_)

    with tile.TileContext(nc) as tc:
        with tc.tile_pool(name="pin", bufs=1) as pin, \
             tc.tile_pool(name="pp", bufs=8) as pp, \
             tc.tile_pool(name="po", bufs=3) as po, \
             tc.tile_pool(name="psum_s", bufs=4, space="PSUM") as psum_s, \
             tc.tile_pool(name="psum_o", bufs=2, space="PSUM") as psum_o, \
             (tc.For_i(0, loop_n, 1) if loop_n else
              contextlib.nullcontext()):

            ident_t = pin.tile([CHUNK, CHUNK], f32r, tag="ident")
            _dma(nc.sync, ident_t[:], id_d[:])
            m4all_t = pin.tile([CHUNK, max(len(slots), 1) * 512], f32r,
                               tag="m4")
            _dma(nc.sync, m4all_t[:], m4_d[:])
            m4_t = {s: m4all_t[:, mi * 512:(mi + 1) * 512]
                    for mi, s in enumerate(slots)}

            # inputs, emitted in compute-consumption order so the first
            # segment's tiles land first and compute starts early
            kT_t = {}
            vO_t = {}
            qT_t = {}
            for i in segs:
                for kv_i in range(KV_PER_CORE):
                    ci0 = chunk_idx[(i, kv_i, 0)]
                    kk = K[i]
                    kt = pin.tile([D, kk * CHUNK], f32, tag=f"kT_{i}_{kv_i}")
                    _dma(nc.sync, kt[:], kT_d[:, ci0 * CHUNK:(ci0 + kk) * CHUNK])
                    kT_t[(i, kv_i)] = kt
                    vt = pin.tile([CHUNK, kk * 130], f32, tag=f"vO_{i}_{kv_i}")
                    _dma(nc.sync, vt[:], vO_d[:, ci0 * 130:(ci0 + kk) * 130])
                    vO_t[(i, kv_i)] = vt
                    si0 = slab_idx[(i, kv_i, 0)]
                    qt = pin.tile([D, kk * 512], f32, tag=f"qT_{i}_{kv_i}")
                    _dma(nc.sync, qt[:], qT_d[:, si0 * 512:(si0 + kk) * 512])
                    for j in range(kk):
                        qT_t[(i, kv_i, j)] = qt[:, j * 512:(j + 1) * 512]

            for i in segs:
                for kv_i in range(KV_PER_CORE):
                    kt = kT_t[(i, kv_i)]
                    vt = vO_t[(i, kv_i)]
                    kk = K[i]
                    ostage = po.tile([CHUNK, kk * 512], f32,
                                     tag=f"os_{i}_{kv_i}", bufs=2)
                    for j in range(kk):
                        qt = qT_t[(i, kv_i, j)]
                        # two 1-bank output tiles (2 heads each) -> can
                        # double-buffer across j iterations
                        ot = [psum_o.tile([CHUNK, 2, 132], f32, tag=f"ot{h}",
                                          name=f"ot{h}")
                              for h in range(2)]
                        pts = []
                        for g0 in range(0, j + 1, GRP):
                            grp = list(range(g0, min(g0 + GRP, j + 1)))
                            slab = psum_s.tile([CHUNK, GRP, 512], f32, tag="slab")
                            for gi, c in enumerate(grp):
                                lhsT = kt[:, c * CHUNK:(c + 1) * CHUNK]
                                masked = (i, c, j) in slot_idx
                                nc.tensor.matmul(
                                    slab[:, gi, :], lhsT, qt,
                                    start=True, stop=not masked)
                                if masked:
                                    nc.tensor.matmul(
                                        slab[:, gi, :], ident_t[:],
                                        m4_t[(i, c, j)],
                                        start=False, stop=True)
                            pt = pp.tile([CHUNK, GRP, 512], f32, tag="pt")
                            nc.scalar.activation(
                                out=pt[:, :len(grp), :],
                                in_=slab[:, :len(grp), :],
                                func=mybir.ActivationFunctionType.Exp)
                            pts.append(pt)
                        for c in range(j + 1):
                            pt = pts[c // GRP]
                            psl = pt[:, c % GRP, :]
                            vsl = vt[:, c * 130:(c + 1) * 130]
                            for g in range(G):
                                # each ot bank holds two heads but forms ONE
                                # accumulation group: start clears has_written
                                # bank-wide, so only the first matmul into the
                                # bank starts and only the last one stops
                                nc.tensor.matmul(
                                    ot[g // 2][:, g % 2, 0:130],
                                    psl[:, g * CHUNK:(g + 1) * CHUNK],
                                    vsl,
                                    start=(c == 0 and g % 2 == 0),
                                    stop=(c == j and g % 2 == 1))
                        recip = po.tile([CHUNK, G], f32, tag="recip")
                        osl = ostage[:, j * 512:(j + 1) * 512]
                        for h in range(2):
                            rh = recip[:, 2 * h:2 * h + 2]
                            nc.vector.reciprocal(out=rh, in_=ot[h][:, :, D])
                            recip_b = bass.AP(
                                tensor=rh.tensor, offset=rh.offset,
                                ap=[rh.ap[0], rh.ap[1], [0, D]])
                            nc.vector.tensor_mul(
                                out=osl[:, 2 * h * 128:(2 * h + 2) * 128]
                                    .rearrange("p (g d) -> p g d", g=2),
                                in0=ot[h][:, :, 0:D],
                                in1=recip_b)
                    si0 = slab_idx[(i, kv_i, 0)]
                    if tiny_dma:
                        nc.sync.dma_start(out=out_d[si0][:1, :4],
                                          in_=ostage[:1, :4])
                    else:
                        nc.sync.dma_start(
                            out=out_d[si0:si0 + kk].rearrange("k p c -> p k c"),
                            in_=ostage[:].rearrange("p (k c) -> p k c", k=kk))

    nc.finalize()
    return nc


# --------------------------------------------------------------------------
# entry point
# --------------------------------------------------------------------------

def kernel(query, key, value, decoder_segment_ids, _trace=False, _numpy=False):
    query = np.asarray(query, np.float32)
    key = np.asarray(key, np.float32)
    value = np.asarray(value, np.float32)
    ids = np.asarray(decoder_segment_ids)
    # the block-diagonal decomposition relies on segment ids being sorted
    # (contiguous segments), as setup_inputs guarantees
    assert np.all(np.diff(ids.astype(np.int64), axis=-1) >= 0)

    runs, L, K, slots, segs, slabs, chunks = _structure(ids)
    core_ins = [_prepare_core(c, query, key, value, runs, L, K, slots,
                              segs, slabs, chunks) for c in range(NCORES)]

    if _numpy:
        outs = [_numpy_schedule(ci, L, K, slots, segs, slabs, chunks)
                for ci in core_ins]
        return _assemble(outs, runs, L, K, slabs)

    from concourse.bass_utils import run_bass_kernel_spmd

    cache_key = (tuple(L), tuple(slots))
    if cache_key not in _PROGRAM_CACHE:
        _PROGRAM_CACHE[cache_key] = _build_program(L, K, slots, segs, slabs,
                                                   chunks)
    nc = _PROGRAM_CACHE[cache_key]

    in_maps = [{k_: v_ for k_, v_ in ci.items()} for ci in core_ins]
    res = run_bass_kernel_spmd(nc, in_maps, list(range(NCORES)), trace=_trace)
    outs = [res.results[c]["out"] for c in range(NCORES)]
    full = _assemble(outs, runs, L, K, slabs)
    if _trace:
        return full, res
    return full



# revision 2
# speedup vs baseline: 2.5036x; 2.5036x over previous
"""Sharded GQA attention (causal + packed-segment mask) for 8 Trainium2 NeuronCores.

Strategy
--------
* Core c handles batch b = c//4 and KV heads {2*(c%4), 2*(c%4)+1} (8 query
  heads per core); the sequence dim stays unsharded.
* decoder_segment_ids are sorted, so the segment mask makes attention
  block-diagonal over contiguous segment spans.  The host reads the actual
  ids, splits each batch into runs, and the device kernel does causal-only
  attention per segment.  The two batches' run structures are unioned
  (padded) so all 8 cores execute one SPMD program; padded "ghost" rows
  contribute nothing (zero K columns give exp(0)=1 but the matching V rows
  and their ones-column are zero, so numerator and denominator are
  unaffected), and ghost query columns produce garbage the host discards.
* Dtypes are chosen per engine roofline: Q/K/mask/ident are fp16 (PE runs
  fp16 at 1 cycle/row vs 4 for fp32; fp16's 10-bit mantissa matches the
  TF32-style rounding fp32r applies anyway), P=exp(S) is written as bf16
  by ScalarE (needs bf16 range: logits are unnormalized, exp can reach
  ~1e27) and V is bf16 to match, with an appended ones column so the
  softmax denominator falls out of the same PV matmuls.  PSUM accumulation
  is fp32 throughout; output is stored fp16.
* Only diagonal chunks need masking (pure causal, one shared [128,512]
  additive tile applied via an identity-stationary matmul); the final
  normalize is a reciprocal + broadcast multiply on DVE fused with the
  PSUM->SBUF copy.  GRP=2 s-chunks share one PSUM slab so each ScalarE
  exp instruction covers 1024 columns, halving fixed access overhead.
"""

import math

import numpy as np
import ml_dtypes

B, T, NQ, NKV, D = 2, 1024, 32, 8, 128
G = NQ // NKV
NCORES = 8
KV_PER_CORE = NKV // (NCORES // B)
CHUNK = 128
NEG = -30000.0  # fp16-safe; exp(NEG + max_logit) == 0 in fp32
GRP = 2  # s-chunks per PSUM slab (2 banks, double-buffered)

F16 = np.float16
BF16 = ml_dtypes.bfloat16

_PROGRAM_CACHE = {}


# --------------------------------------------------------------------------
# host-side structure
# --------------------------------------------------------------------------

def _runs(seg_row):
    d = np.flatnonzero(np.diff(seg_row) != 0)
    starts = np.concatenate(([0], d + 1))
    ends = np.concatenate((d + 1, [len(seg_row)]))
    return [(int(s), int(e - s)) for s, e in zip(starts, ends)]


def _structure(ids):
    runs = [_runs(np.asarray(ids[b])) for b in range(B)]
    n_seg = max(len(r) for r in runs)
    L = [max((r[i][1] for r in runs if len(r) > i), default=0) for i in range(n_seg)]
    K = [math.ceil(l / CHUNK) for l in L]
    segs = [i for i in range(n_seg) if K[i] > 0]
    slabs = [(i, kv_i, j) for i in segs for kv_i in range(KV_PER_CORE)
             for j in range(K[i])]
    chunks = [(i, kv_i, c) for i in segs for kv_i in range(KV_PER_CORE)
              for c in range(K[i])]
    return runs, L, K, segs, slabs, chunks


def _prepare_core(core, q, k, v, runs, L, K, segs, slabs, chunks):
    b = core // (NCORES // B)
    kv_heads = [KV_PER_CORE * (core % (NCORES // B)) + x for x in range(KV_PER_CORE)]
    rb = runs[b]

    def seg_info(i):
        if i < len(rb):
            return rb[i]
        return (0, 0)

    qT = np.zeros((D, len(slabs) * 4 * CHUNK), F16)
    for si, (i, kv_i, j) in enumerate(slabs):
        a, lb = seg_info(i)
        t0 = j * CHUNK
        n_real = min(CHUNK, lb - t0)
        if n_real > 0:
            for g in range(G):
                h = G * kv_heads[kv_i] + g
                blk = q[b, a + t0:a + t0 + n_real, h, :]  # [n_real, D]
                qT[:, si * 512 + g * CHUNK: si * 512 + g * CHUNK + n_real] = \
                    blk.T.astype(F16)

    kT = np.zeros((D, len(chunks) * CHUNK), F16)
    vO = np.zeros((CHUNK, len(chunks) * 130), BF16)
    for ci, (i, kv_i, c) in enumerate(chunks):
        a, lb = seg_info(i)
        s0 = c * CHUNK
        n_real = min(CHUNK, lb - s0)
        if n_real > 0:
            kvh = kv_heads[kv_i]
            kT[:, ci * CHUNK: ci * CHUNK + n_real] = \
                k[b, a + s0:a + s0 + n_real, kvh, :].T.astype(F16)
            vO[:n_real, ci * 130: ci * 130 + D] = \
                v[b, a + s0:a + s0 + n_real, kvh, :].astype(BF16)
            vO[:n_real, ci * 130 + D] = BF16(1.0)

    # one shared causal mask tile: NEG strictly below the diagonal
    # (s > t), tiled across the G query heads
    sr = np.arange(CHUNK)
    m = np.where(sr[:, None] > sr[None, :], F16(NEG), F16(0.0))
    m4 = np.tile(m, (1, G))

    return {"qT": qT, "kT": kT, "vO": vO, "m4": m4,
            "ident": np.eye(CHUNK, dtype=F16)}


def _assemble(outs, runs, L, K, slabs):
    full = np.zeros((B, T, NQ, D), np.float32)
    for core in range(NCORES):
        b = core // (NCORES // B)
        kv_heads = [KV_PER_CORE * (core % (NCORES // B)) + x
                    for x in range(KV_PER_CORE)]
        res = outs[core]  # [NSLAB, 128, 512] fp16
        rb = runs[b]
        for si, (i, kv_i, j) in enumerate(slabs):
            if i >= len(rb):
                continue
            a, lb = rb[i]
            t0 = j * CHUNK
            n_real = min(CHUNK, lb - t0)
            if n_real <= 0:
                continue
            for g in range(G):
                h = G * kv_heads[kv_i] + g
                full[b, a + t0:a + t0 + n_real, h, :] = \
                    res[si, :n_real, g * CHUNK:g * CHUNK + D].astype(np.float32)
    return full


# --------------------------------------------------------------------------
# numpy emulation of the device schedule (debug/validation only)
# --------------------------------------------------------------------------

def _numpy_schedule(ins, L, K, segs, slabs, chunks):
    slab_idx = {s: i for i, s in enumerate(slabs)}
    chunk_idx = {c: i for i, c in enumerate(chunks)}
    qT = ins["qT"].astype(np.float32)
    kT = ins["kT"].astype(np.float32)
    vO = ins["vO"].astype(np.float32)
    m4 = ins["m4"].astype(np.float32)
    out = np.zeros((len(slabs), CHUNK, 512), np.float32)
    for i in segs:
        for kv_i in range(KV_PER_CORE):
            for j in range(K[i]):
                si = slab_idx[(i, kv_i, j)]
                ot = np.zeros((CHUNK, G, 129), np.float32)
                for c in range(j + 1):
                    ci = chunk_idx[(i, kv_i, c)]
                    lhsT = kT[:, ci * CHUNK:(ci + 1) * CHUNK]          # [d, s]
                    rhs = qT[:, si * 512:(si + 1) * 512]               # [d, (g,t)]
                    S = lhsT.T @ rhs                                   # [s, (g,t)]
                    if c == j:
                        S = S + m4
                    P = np.exp(S).astype(BF16).astype(np.float32)
                    vo = vO[:, ci * 130:ci * 130 + 129]                # [s, 129]
                    for g in range(G):
                        ot[:, g, :] += P[:, g * CHUNK:(g + 1) * CHUNK].T @ vo
                denom = ot[:, :, D:D + 1]
                with np.errstate(divide="ignore", invalid="ignore"):
                    norm = ot[:, :, :D] / denom
                out[si] = norm.reshape(CHUNK, G * D)
    return out.astype(F16)


# --------------------------------------------------------------------------
# bass program
# --------------------------------------------------------------------------

def _build_program(L, K, segs, slabs, chunks, loop_n=0):
    import contextlib

    import concourse.bacc as bacc
    import concourse.bass as bass
    import concourse.tile as tile
    from concourse import mybir

    slab_idx = {s: i for i, s in enumerate(slabs)}
    chunk_idx = {c: i for i, c in enumerate(chunks)}
    f32 = mybir.dt.float32
    f16 = mybir.dt.float16
    bf16 = mybir.dt.bfloat16

    nc = bacc.Bacc()
    qT_d = nc.dram_tensor("qT", [D, len(slabs) * 512], f16, kind="ExternalInput")
    kT_d = nc.dram_tensor("kT", [D, len(chunks) * CHUNK], f16, kind="ExternalInput")
    vO_d = nc.dram_tensor("vO", [CHUNK, len(chunks) * 130], bf16,
                          kind="ExternalInput")
    m4_d = nc.dram_tensor("m4", [CHUNK, 512], f16, kind="ExternalInput")
    id_d = nc.dram_tensor("ident", [CHUNK, CHUNK], f16, kind="ExternalInput")
    out_d = nc.dram_tensor("out", [len(slabs), CHUNK, 512], f16,
                           kind="ExternalOutput")

    with tile.TileContext(nc) as tc:
        with tc.tile_pool(name="pin", bufs=1) as pin, \
             tc.tile_pool(name="pp", bufs=8) as pp, \
             tc.tile_pool(name="po", bufs=2) as po, \
             tc.tile_pool(name="psum_s", bufs=2, space="PSUM") as psum_s, \
             tc.tile_pool(name="psum_o", bufs=2, space="PSUM") as psum_o, \
             (tc.For_i(0, loop_n, 1) if loop_n else
              contextlib.nullcontext()):

            ident_t = pin.tile([CHUNK, CHUNK], f16, tag="ident")
            nc.sync.dma_start(out=ident_t[:], in_=id_d[:])
            m4_t = pin.tile([CHUNK, 512], f16, tag="m4")
            nc.sync.dma_start(out=m4_t[:], in_=m4_d[:])

            # inputs, emitted in compute-consumption order so the first
            # segment's tiles land first and compute starts early
            kT_t = {}
            vO_t = {}
            qT_t = {}
            for i in segs:
                for kv_i in range(KV_PER_CORE):
                    ci0 = chunk_idx[(i, kv_i, 0)]
                    kk = K[i]
                    kt = pin.tile([D, kk * CHUNK], f16, tag=f"kT_{i}_{kv_i}")
                    nc.sync.dma_start(out=kt[:],
                                      in_=kT_d[:, ci0 * CHUNK:(ci0 + kk) * CHUNK])
                    kT_t[(i, kv_i)] = kt
                    vt = pin.tile([CHUNK, kk * 130], bf16, tag=f"vO_{i}_{kv_i}")
                    nc.sync.dma_start(out=vt[:],
                                      in_=vO_d[:, ci0 * 130:(ci0 + kk) * 130])
                    vO_t[(i, kv_i)] = vt
                    si0 = slab_idx[(i, kv_i, 0)]
                    qt = pin.tile([D, kk * 512], f16, tag=f"qT_{i}_{kv_i}")
                    nc.sync.dma_start(out=qt[:],
                                      in_=qT_d[:, si0 * 512:(si0 + kk) * 512])
                    for j in range(kk):
                        qT_t[(i, kv_i, j)] = qt[:, j * 512:(j + 1) * 512]

            for i in segs:
                for kv_i in range(KV_PER_CORE):
                    kt = kT_t[(i, kv_i)]
                    vt = vO_t[(i, kv_i)]
                    kk = K[i]
                    ostage = po.tile([CHUNK, kk * 512], f16, tag="ostage")
                    for j in range(kk):
                        qt = qT_t[(i, kv_i, j)]
                        # two 1-bank output tiles (2 heads each) -> can
                        # double-buffer across j iterations
                        ot = [psum_o.tile([CHUNK, 2, 132], f32, tag=f"ot{h}",
                                          name=f"ot{h}")
                              for h in range(2)]
                        pts = []
                        for g0 in range(0, j + 1, GRP):
                            grp = list(range(g0, min(g0 + GRP, j + 1)))
                            slab = psum_s.tile([CHUNK, GRP, 512], f32, tag="slab")
                            for gi, c in enumerate(grp):
                                lhsT = kt[:, c * CHUNK:(c + 1) * CHUNK]
                                masked = c == j
                                nc.tensor.matmul(
                                    slab[:, gi, :], lhsT, qt,
                                    start=True, stop=not masked)
                                if masked:
                                    nc.tensor.matmul(
                                        slab[:, gi, :], ident_t[:], m4_t[:],
                                        start=False, stop=True)
                            pt = pp.tile([CHUNK, GRP, 512], bf16, tag="pt")
                            nc.scalar.activation(
                                out=pt[:, :len(grp), :],
                                in_=slab[:, :len(grp), :],
                                func=mybir.ActivationFunctionType.Exp)
                            pts.append(pt)
                        for c in range(j + 1):
                            pt = pts[c // GRP]
                            psl = pt[:, c % GRP, :]
                            vsl = vt[:, c * 130:(c + 1) * 130]
                            for g in range(G):
                                # each ot bank holds two heads but forms ONE
                                # accumulation group: start clears has_written
                                # bank-wide, so only the first matmul into the
                                # bank starts and only the last one stops
                                nc.tensor.matmul(
                                    ot[g // 2][:, g % 2, 0:130],
                                    psl[:, g * CHUNK:(g + 1) * CHUNK],
                                    vsl,
                                    start=(c == 0 and g % 2 == 0),
                                    stop=(c == j and g % 2 == 1))
                        recip = po.tile([CHUNK, G], f32, tag="recip")
                        osl = ostage[:, j * 512:(j + 1) * 512]
                        for h in range(2):
                            rh = recip[:, 2 * h:2 * h + 2]
                            nc.vector.reciprocal(out=rh, in_=ot[h][:, :, D])
                            recip_b = bass.AP(
                                tensor=rh.tensor, offset=rh.offset,
                                ap=[rh.ap[0], rh.ap[1], [0, D]])
                            nc.vector.tensor_mul(
                                out=osl[:, 2 * h * 128:(2 * h + 2) * 128]
                                    .rearrange("p (g d) -> p g d", g=2),
                                in0=ot[h][:, :, 0:D],
                                in1=recip_b)
                    si0 = slab_idx[(i, kv_i, 0)]
                    nc.sync.dma_start(
                        out=out_d[si0:si0 + kk].rearrange("k p c -> p k c"),
                        in_=ostage[:].rearrange("p (k c) -> p k c", k=kk))

    nc.finalize()
    return nc


# --------------------------------------------------------------------------
# entry point
# --------------------------------------------------------------------------

def kernel(query, key, value, decoder_segment_ids, _trace=False, _numpy=False):
    query = np.asarray(query, np.float32)
    key = np.asarray(key, np.float32)
    value = np.asarray(value, np.float32)
    ids = np.asarray(decoder_segment_ids)
    # the block-diagonal decomposition relies on segment ids being sorted
    # (contiguous segments), as setup_inputs guarantees
    assert np.all(np.diff(ids.astype(np.int64), axis=-1) >= 0)

    runs, L, K, segs, slabs, chunks = _structure(ids)
    core_ins = [_prepare_core(c, query, key, value, runs, L, K,
                              segs, slabs, chunks) for c in range(NCORES)]

    if _numpy:
        outs = [_numpy_schedule(ci, L, K, segs, slabs, chunks)
                for ci in core_ins]
        return _assemble(outs, runs, L, K, slabs)

    from concourse.bass_utils import run_bass_kernel_spmd

    cache_key = tuple(L)
    if cache_key not in _PROGRAM_CACHE:
        _PROGRAM_CACHE[cache_key] = _build_program(L, K, segs, slabs, chunks)
    nc = _PROGRAM_CACHE[cache_key]

    in_maps = [{k_: v_ for k_, v_ in ci.items()} for ci in core_ins]
    res = run_bass_kernel_spmd(nc, in_maps, list(range(NCORES)), trace=_trace)
    outs = [res.results[c]["out"] for c in range(NCORES)]
    full = _assemble(outs, runs, L, K, slabs)
    if _trace:
        return full, res
    return full
